# revision 13
# baseline (speedup 1.0000x reference)
"""LID detector kernel for Trainium2 (8 NeuronCores, data-parallel over batch).

Per core (batch shard of 32 samples):
  - features arrive as fp8(e4m3); mean-pool over space on DVE/ACT/GPSIMD
    into f32 sums, scaled into fp16 qT tiles (holding 2*q)
  - refs arrive as fp16 [C, R] (host-transposed; L2/L3 rows permuted to match
    the on-chip channel interleave — distances are invariant to a shared
    channel permutation of q and ref)
  - big[row,(l,s), ref] = 2q.r - ||r||^2 accumulated by fp16 PE matmuls into
    one [128, 2000] PSUM tile: dist matmuls (lhsT=qT) plus fold matmuls
    (lhsT=-1s, rhs=sq=rt^2).  L2+L3 share the base-64 quadrant with
    zero-padded lhsT column blocks (matmul PSUM base partition must be
    0/32/64).
  - top-24 via DVE max8/match_replace straight on PSUM, split into two
    column halves + a 48-wide merge so half A overlaps half B's matmuls
  - vals += -qn2 per row -> -d2; LID = -2k/(sum ln d2_i - 20 ln d2_20);
    4->1 regression; sigmoid
"""

import sys

for _p in ("/opt/trn_rl_repo", "/root/.axon_site/_ro/trn_rl_repo"):
    if _p not in sys.path:
        sys.path.append(_p)

import ml_dtypes
import numpy as np

import concourse.mybir as mybir
from concourse import bass, bacc
from concourse.tile import TileContext
from concourse.bass_utils import run_bass_kernel_spmd

F32 = mybir.dt.float32
F16 = mybir.dt.float16
F8 = mybir.dt.float8e4
N_CORES = 8
B = 32
R = 2000
K = 20
NEG_BIG = -3.0e38
ALU = mybir.AluOpType
ACTF = mybir.ActivationFunctionType
AX = mybir.AxisListType

# (C, HW, channels-per-partition)
LAYERS = [(64, 3136, 1), (128, 784, 1), (256, 196, 2), (512, 49, 4)]

# --- engine assignment knobs -----------------------------------------------
# L0: 16 pairs (2 samples x 64ch on 128 partitions, 3136 free cols each).
#   'A' ACT accum | 'V' DVE reduce | 'GV'/'GA' gpsimd halve + DVE/ACT tail
L0_ASSIGN = ["GV", "A", "GV", "A", "GV", "GV", "A", "GV",
             "A", "GV", "GV", "A", "GV", "A", "GV", "GV"]  # 10 GV, 6 A
# L1: 8 tiles [128, 4, 784]. 'V' | 'A' (4 ops) | 'GV'
L1_ASSIGN = ["V", "A", "V", "A", "V", "GV", "V", "A",
             "V", "A", "V", "GV", "V", "A", "V", "A"]  # 8V, 6A, 2GV
# L2: 8 tiles [128, 4, 2, 196]. 'V' | 'GV'
L2_ASSIGN = ["V", "GV", "GV", "GV", "V", "GV", "GV", "GV"]
# L3: 8 tiles [128, 4, 4, 49]. 'V' | 'GV'
L3_ASSIGN = ["V", "GV", "V", "GV", "V", "GV", "V", "GV"]
# squares, keyed (layer, chunk_offset): 'A' | 'V' | 'G'
SQ_EARLY = [(2, 0), (2, 128), (3, 0), (3, 128), (3, 256), (3, 384)]
SQ_LATE = [(0, 0), (1, 0)]

RCHUNKS = [(0, 512), (512, 512), (1024, 512), (1536, 464)]
HALF_A = (0, 1024)
HALF_B = (1024, 976)


def build_nc():
    nc = bacc.Bacc("TRN2", target_bir_lowering=False, debug=False,
                   num_devices=N_CORES)

    feats = [nc.dram_tensor(f"feat{l}", [B, C, HW], F8, kind="ExternalInput")
             for l, (C, HW, _) in enumerate(LAYERS)]
    refTs = [nc.dram_tensor(f"refT{l}", [C, R], F16, kind="ExternalInput")
             for l, (C, _, _) in enumerate(LAYERS)]
    regw = nc.dram_tensor("regw", [1, 4], F32, kind="ExternalInput")
    regb = nc.dram_tensor("regb", [1, 1], F32, kind="ExternalInput")
    out = nc.dram_tensor("out", [B, 1], F32, kind="ExternalOutput")

    with TileContext(nc) as tc:
        with (
            tc.tile_pool(name="persist", bufs=1) as pp,
            tc.tile_pool(name="ft0", bufs=6) as fp0,
            tc.tile_pool(name="ft1", bufs=16) as fp1,
            tc.tile_pool(name="ft23", bufs=8) as fp23,
            tc.tile_pool(name="half", bufs=3) as hvp,
            tc.tile_pool(name="pbig", bufs=1,
                         space=bass.MemorySpace.PSUM) as pbig,
            tc.tile_pool(name="psmall", bufs=1,
                         space=bass.MemorySpace.PSUM) as psml,
        ):
            # ---------------- persistent tiles
            rt = {}
            sq = {}
            for l, (C, _, _) in enumerate(LAYERS):
                for i in range(0, C, 128):
                    Cc = min(128, C - i)
                    rt[(l, i)] = pp.tile([Cc, R], F16, tag=f"rt{l}_{i}",
                                         name=f"rt{l}_{i}")
                    sq[(l, i)] = pp.tile([Cc, R], F16, tag=f"sq{l}_{i}",
                                         name=f"sq{l}_{i}")
            sums0 = pp.tile([128, 16], F32, tag="sums0", name="sums0")
            sums1 = pp.tile([128, B], F32, tag="sums1", name="sums1")
            sums2 = pp.tile([128, B, 2], F32, tag="sums2", name="sums2")
            sums3 = pp.tile([128, B, 4], F32, tag="sums3", name="sums3")
            qT0 = pp.tile([64, 16, 2], F16, tag="qT0", name="qT0")
            qT1 = pp.tile([128, B], F16, tag="qT1", name="qT1")
            qT23 = {}
            for l, nch in ((2, 2), (3, 4)):
                for j in range(nch):
                    qT23[(l, j)] = pp.tile([128, 64], F16, tag=f"qT{l}_{j}",
                                           name=f"qT{l}_{j}")
            ones_col = pp.tile([128, 1], F16, tag="ones_col", name="ones_col")
            ones_row = pp.tile([1, B], F32, tag="ones_row", name="ones_row")
            # fold lhsT masks: row usage via column slices
            neg_full = pp.tile([128, 64], F16, tag="neg_full", name="neg_full")
            neg_l2 = pp.tile([128, 64], F16, tag="neg_l2", name="neg_l2")
            neg_l3 = pp.tile([128, 64], F16, tag="neg_l3", name="neg_l3")
            wb_sb = pp.tile([1, 5], F32, tag="wb_sb", name="wb_sb")
            vals48 = pp.tile([128, 48], F32, tag="vals48", name="vals48")
            vals = pp.tile([128, 24], F32, tag="vals", name="vals")
            fixrow = pp.tile([128, 1], F32, tag="fixrow", name="fixrow")
            scr0 = pp.tile([128, 3136], F8, tag="scr0", name="scr0")
            scr0b = pp.tile([128, 1568], F16, tag="scr0b", name="scr0b")
            scr1 = pp.tile([128, 784], F8, tag="scr1", name="scr1")

            nc.vector.memset(ones_col[:], 1.0)
            nc.vector.memset(ones_row[:], 1.0)
            nc.vector.memset(neg_full[:], -1.0)
            nc.vector.memset(neg_l2[:, 0:32], -1.0)
            nc.vector.memset(neg_l2[:, 32:64], 0.0)
            nc.vector.memset(neg_l3[:, 0:32], 0.0)
            nc.vector.memset(neg_l3[:, 32:64], -1.0)
            for l, nch in ((2, 2), (3, 4)):
                zs = slice(32, 64) if l == 2 else slice(0, 32)
                for j in range(nch):
                    nc.vector.memset(qT23[(l, j)][:, zs], 0.0)
            nc.sync.dma_start(out=wb_sb[0:1, 0:4], in_=regw[:])
            nc.sync.dma_start(out=wb_sb[0:1, 4:5], in_=regb[:])

            # ---------------- DMAs: L2/L3 + refs first, then L0, then L1
            # (small layers pool early so their long distance-matmul chains
            # overlap L0/L1 pooling; L0/L1 dists at the end are short)
            ref_order = [(3, 0), (3, 128), (3, 256), (3, 384),
                         (2, 0), (2, 128), (1, 0), (0, 0)]
            ftiles = {}

            def dma_feat(l, t):
                C, HW, cpp = LAYERS[l]
                nS = 2 if l in (0, 1) else 4
                if l == 0:
                    tile = fp0.tile([128, nS, HW], F8, tag="f0",
                                    name=f"f0_{t}")
                    src = bass.AP(feats[0], 4 * t * C * HW,
                                  [[HW, 128], [2 * C * HW, nS], [1, HW]])
                else:
                    fpool = fp1 if l == 1 else fp23
                    tile = fpool.tile([128, nS, cpp, HW], F8, tag=f"f{l}",
                                      name=f"f{l}_{t}")
                    src = bass.AP(feats[l], nS * t * C * HW,
                                  [[cpp * HW, 128], [C * HW, nS],
                                   [HW, cpp], [1, HW]])
                nc.sync.dma_start(out=tile[:], in_=src)
                ftiles[(l, t)] = tile

            for t in range(8):
                dma_feat(2, t)
                li = ref_order[t]
                nc.sync.dma_start(
                    out=rt[li][:],
                    in_=refTs[li[0]][li[1]:li[1] + rt[li].shape[0], :])
            for t in range(8):
                dma_feat(3, t)
            for t in range(8):
                dma_feat(0, t)
                dma_feat(1, 2 * t)
                dma_feat(1, 2 * t + 1)

            # ---------------- squares (sq = rt^2, fp16) on ACT
            # 6 early chunks fill ACT's idle window before L0 tiles arrive
            for (l, i) in SQ_EARLY:
                nc.scalar.square(sq[(l, i)][:], rt[(l, i)][:])

            # ---------------- fold matmuls first: big = -sum_c r_c^2
            # (depend only on refs/squares -> PE busy during pooling)
            big = pbig.tile([128, R], F32, tag="big", name="big")
            for c0, n in RCHUNKS:
                cs = slice(c0, c0 + n)
                for j in range(4):
                    i = 128 * j
                    nc.tensor.matmul(big[64:128, cs], neg_l3[:],
                                     sq[(3, i)][:, cs],
                                     start=(j == 0), stop=False)
                for j in range(2):
                    i = 128 * j
                    nc.tensor.matmul(big[64:128, cs], neg_l2[:],
                                     sq[(2, i)][:, cs],
                                     start=False, stop=False)
                nc.tensor.matmul(big[32:64, cs], neg_full[:, 0:32],
                                 sq[(1, 0)][:, cs], start=True, stop=False)
                nc.tensor.matmul(big[0:32, cs], neg_full[0:64, 0:32],
                                 sq[(0, 0)][:, cs], start=True, stop=False)

            # ---------------- pooling: L2, L3 first, then L0, then L1
            # L2 / L3
            for l in (2, 3):
                C, HW, cpp = LAYERS[l]
                assign = L2_ASSIGN if l == 2 else L3_ASSIGN
                sums = sums2 if l == 2 else sums3
                for t in range(8):
                    tile = ftiles[(l, t)]
                    eng = assign[t]
                    dst = sums[:, 4 * t:4 * t + 4, :]
                    if eng == "V":
                        nc.vector.tensor_reduce(dst, tile[:], axis=AX.X,
                                                op=ALU.add)
                    else:
                        hw2 = HW // 2  # 98 | 24 (49 odd: copy tail col)
                        if l == 2:
                            h = hvp.tile([128, 4, cpp, hw2], F16,
                                         tag=f"h{l}", name=f"h{l}_{t}")
                            nc.gpsimd.tensor_tensor(
                                h[:], tile[:, :, :, 0:hw2],
                                tile[:, :, :, hw2:HW], op=ALU.add)
                            nc.vector.tensor_reduce(dst, h[:], axis=AX.X,
                                                    op=ALU.add)
                        else:
                            h = hvp.tile([128, 4, cpp, hw2 + 1], F16,
                                         tag=f"h{l}", name=f"h{l}_{t}")
                            nc.gpsimd.tensor_tensor(
                                h[:, :, :, 0:hw2], tile[:, :, :, 0:hw2],
                                tile[:, :, :, hw2:2 * hw2], op=ALU.add)
                            nc.gpsimd.tensor_copy(h[:, :, :, hw2:hw2 + 1],
                                                  tile[:, :, :, 2 * hw2:HW])
                            nc.vector.tensor_reduce(dst, h[:], axis=AX.X,
                                                    op=ALU.add)
                for j in range(cpp):
                    nc.scalar.mul(qT23[(l, j)][:, 0:32] if l == 2
                                  else qT23[(l, j)][:, 32:64],
                                  sums[:, :, j], 2.0 / HW)

            # ---------------- quadrant-2 distances (overlap L0/L1 pooling)
            def dist_q2(c0, n, stop):
                cs = slice(c0, c0 + n)
                for l, nch in ((2, 2), (3, 4)):
                    for j in range(nch):
                        nc.tensor.matmul(
                            big[64:128, cs], qT23[(l, j)][:],
                            rt[(l, 128 * j)][:, cs], start=False,
                            stop=(stop and l == 3 and j == nch - 1))

            for c0, n in RCHUNKS:
                dist_q2(c0, n, True)

            # late squares (needed only by Q0/Q1 folds near the end)
            for (l, i) in SQ_LATE:
                nc.scalar.square(sq[(l, i)][:], rt[(l, i)][:])

            # L0 pairs + L1 half-tiles, interleaved per DMA round
            C0, HW0, _ = LAYERS[0]
            C1, HW1, _ = LAYERS[1]

            def pool_l0(pr):
                tile = ftiles[(0, pr // 2)]
                u = pr % 2
                eng = L0_ASSIGN[pr]
                dst = sums0[:, pr:pr + 1]
                if eng == "A":
                    nc.scalar.activation(scr0[:, :], tile[:, u, :],
                                         ACTF.Copy, accum_out=dst)
                elif eng == "V":
                    nc.vector.tensor_reduce(dst, tile[:, u, :],
                                            axis=AX.X, op=ALU.add)
                else:
                    h = hvp.tile([128, HW0 // 2], F16, tag="h0",
                                 name=f"h0_{pr}")
                    nc.gpsimd.tensor_tensor(
                        h[:], tile[:, u, 0:HW0 // 2],
                        tile[:, u, HW0 // 2:HW0], op=ALU.add)
                    if eng == "GV":
                        nc.vector.tensor_reduce(dst, h[:], axis=AX.X,
                                                op=ALU.add)
                    else:
                        nc.scalar.activation(scr0b[:, :], h[:],
                                             ACTF.Copy, accum_out=dst)

            def pool_l1(t):
                tile = ftiles[(1, t)]
                eng = L1_ASSIGN[t]
                dst = sums1[:, 2 * t:2 * t + 2]
                if eng == "V":
                    nc.vector.tensor_reduce(dst, tile[:, :, 0, :],
                                            axis=AX.X, op=ALU.add)
                elif eng == "A":
                    for g in range(2):
                        nc.scalar.activation(scr1[:, :], tile[:, g, 0, :],
                                             ACTF.Copy,
                                             accum_out=dst[:, g:g + 1])
                else:
                    h = hvp.tile([128, 2, HW1 // 2], F16, tag="h1",
                                 name=f"h1_{t}")
                    nc.gpsimd.tensor_tensor(h[:], tile[:, :, 0, 0:HW1 // 2],
                                            tile[:, :, 0, HW1 // 2:HW1],
                                            op=ALU.add)
                    nc.vector.tensor_reduce(dst, h[:], axis=AX.X, op=ALU.add)

            for t in range(8):
                pool_l0(2 * t)
                pool_l0(2 * t + 1)
                pool_l1(2 * t)
                pool_l1(2 * t + 1)
            for h in range(2):
                nc.scalar.mul(qT0[:, :, h], sums0[64 * h:64 * h + 64, :],
                              2.0 / HW0)
            nc.scalar.mul(qT1[:], sums1[:], 2.0 / HW1)
            for c0, n in RCHUNKS:
                cs = slice(c0, c0 + n)
                nc.tensor.matmul(big[0:32, cs], qT0[:], rt[(0, 0)][:, cs],
                                 start=False, stop=True)
                nc.tensor.matmul(big[32:64, cs], qT1[:], rt[(1, 0)][:, cs],
                                 start=False, stop=True)

            # ---------------- top-24 (single pass over psum)
            nc.vector.max(vals[:, 0:8], big[:])
            nc.vector.match_replace(big[:], vals[:, 0:8], big[:], NEG_BIG)
            nc.vector.max(vals[:, 8:16], big[:])
            nc.vector.match_replace(big[:], vals[:, 8:16], big[:], NEG_BIG)
            nc.vector.max(vals[:, 16:24], big[:])

            # ---------------- qn2 -> fixrow = -qn2 per (l, s) row
            qsq0 = pp.tile([64, 16, 2], F16, tag="qsq0", name="qsq0")
            qsq1 = pp.tile([128, B], F16, tag="qsq1", name="qsq1")
            nc.scalar.activation(qsq0[:], qT0[:], ACTF.Square, scale=0.5)
            nc.scalar.activation(qsq1[:], qT1[:], ACTF.Square, scale=0.5)
            qpsA = psml.tile([64, 1], F32, tag="qpsA", name="qpsA")
            nc.tensor.matmul(qpsA[0:32, :], qsq0[:], ones_col[0:64, 0:1],
                             start=True, stop=True)
            nc.tensor.matmul(qpsA[32:64, :], qsq1[:], ones_col[:, 0:1],
                             start=True, stop=True)
            qpsB = psml.tile([64, 1], F32, tag="qpsB", name="qpsB")
            first = True
            for l, nch in ((2, 2), (3, 4)):
                for j in range(nch):
                    qsq = pp.tile([128, 64], F16, tag=f"qsq{l}_{j}",
                                  name=f"qsq{l}_{j}")
                    nc.scalar.activation(qsq[:], qT23[(l, j)][:],
                                         ACTF.Square, scale=0.5)
                    nc.tensor.matmul(qpsB[:], qsq[:], ones_col[:, 0:1],
                                     start=first,
                                     stop=(l == 3 and j == nch - 1))
                    first = False
            for l in range(4):
                src = qpsA if l < 2 else qpsB
                r0 = 32 * (l % 2)
                nc.scalar.activation(fixrow[32 * l:32 * l + 32, :],
                                     src[r0:r0 + 32, :], ACTF.Copy,
                                     scale=-1.0)

            # ---------------- LID
            ln2 = pp.tile([128, 24], F32, tag="ln2", name="ln2")
            S = pp.tile([128, 1], F32, tag="S", name="S")
            denom = pp.tile([128, 1], F32, tag="denom", name="denom")
            lid = pp.tile([128, 1], F32, tag="lid", name="lid")
            nc.vector.tensor_scalar(vals[:], vals[:], fixrow[:], -1e-30,
                                    op0=ALU.add, op1=ALU.min)
            nc.scalar.activation(ln2[:], vals[:], ACTF.Ln, scale=-1.0)
            nc.vector.tensor_reduce(S[:], ln2[:, 1:21], axis=AX.X,
                                    op=ALU.add)
            nc.vector.tensor_scalar(denom[:], ln2[:, 20:21], -20.0, S[:],
                                    op0=ALU.mult, op1=ALU.add)
            # denom scaled by -1/(2K) during the fuse above would change S;
            # keep DVE reciprocal then fold -2K via the lid4 copies' source
            nc.vector.reciprocal(lid[:], denom[:])
            nc.vector.tensor_scalar_mul(lid[:], lid[:], -2.0 * K)

            # ---------------- regression + sigmoid
            lid4 = pp.tile([B, 4], F32, tag="lid4", name="lid4")
            for l in range(4):
                nc.vector.tensor_copy(lid4[:, l:l + 1],
                                      lid[32 * l:32 * l + 32, :])
            wps = psml.tile([B, 5], F32, tag="wps", name="wps")
            nc.tensor.matmul(wps[:], ones_row[:], wb_sb[:],
                             start=True, stop=True)
            wbc = pp.tile([B, 5], F32, tag="wbc", name="wbc")
            nc.scalar.copy(wbc[:], wps[:])
            prod = pp.tile([B, 4], F32, tag="prod", name="prod")
            nc.vector.tensor_tensor(prod[:], lid4[:], wbc[:, 0:4],
                                    op=ALU.mult)
            ssum = pp.tile([B, 1], F32, tag="ssum", name="ssum")
            nc.vector.tensor_reduce(ssum[:], prod[:], axis=AX.X, op=ALU.add)
            res = pp.tile([B, 1], F32, tag="res", name="res")
            nc.scalar.activation(res[:], ssum[:], ACTF.Sigmoid,
                                 bias=wbc[:, 4:5])
            nc.sync.dma_start(out=out[:], in_=res[:])

    nc.compile()
    return nc


_NC = None


def _get_nc():
    global _NC
    if _NC is None:
        _NC = build_nc()
    return _NC


def _perm(cpp, C):
    return [cpp * p + j for j in range(cpp) for p in range(C // cpp)]


def run(trace=False, **inputs):
    nc = _get_nc()
    feats = [np.asarray(inputs[f"feat{l}"], dtype=np.float32)
             for l in range(4)]
    refTs = []
    for l, (C, HW, cpp) in enumerate(LAYERS):
        rT = np.asarray(inputs[f"ref{l}"], dtype=np.float32).T  # [C, R]
        if cpp > 1:
            rT = rT[_perm(cpp, C)]
        refTs.append(np.ascontiguousarray(rT).astype(np.float16))
    regw = np.asarray(inputs["reg_w"], dtype=np.float32).reshape(1, 4)
    regb = np.asarray(inputs["reg_b"], dtype=np.float32).reshape(1, 1)
    assert int(inputs.get("k", K)) == K

    in_maps = []
    for c in range(N_CORES):
        m = {}
        for l, (C, HW, _) in enumerate(LAYERS):
            m[f"feat{l}"] = np.ascontiguousarray(
                feats[l][c * B:(c + 1) * B].reshape(B, C, HW)).astype(
                    ml_dtypes.float8_e4m3)
            m[f"refT{l}"] = refTs[l]
        m["regw"] = regw
        m["regb"] = regb
        in_maps.append(m)

    res = run_bass_kernel_spmd(nc, in_maps, core_ids=list(range(N_CORES)),
                               trace=trace)
    full = np.empty((N_CORES * B,), dtype=np.float32)
    for c in range(N_CORES):
        full[c * B:(c + 1) * B] = res.results[c]["out"][:, 0]
    return full, res


def kernel(**inputs):
    return run(trace=False, **inputs)[0]


# revision 14
# speedup vs baseline: 1.0039x; 1.0039x over previous
"""LID detector kernel for Trainium2 (8 NeuronCores, data-parallel over batch).

Per core (batch shard of 32 samples):
  - features arrive as fp8(e4m3); mean-pool over space on DVE/ACT/GPSIMD
    into f32 sums, scaled into fp16 qT tiles (holding 2*q)
  - refs arrive as fp16 [C, R] (host-transposed; L2/L3 rows permuted to match
    the on-chip channel interleave — distances are invariant to a shared
    channel permutation of q and ref)
  - big[row,(l,s), ref] = 2q.r - ||r||^2 accumulated by fp16 PE matmuls into
    one [128, 2000] PSUM tile: dist matmuls (lhsT=qT) plus fold matmuls
    (lhsT=-1s, rhs=sq=rt^2).  L2+L3 share the base-64 quadrant with
    zero-padded lhsT column blocks (matmul PSUM base partition must be
    0/32/64).
  - top-24 via DVE max8/match_replace straight on PSUM, split into two
    column halves + a 48-wide merge so half A overlaps half B's matmuls
  - vals += -qn2 per row -> -d2; LID = -2k/(sum ln d2_i - 20 ln d2_20);
    4->1 regression; sigmoid
"""

import sys

for _p in ("/opt/trn_rl_repo", "/root/.axon_site/_ro/trn_rl_repo"):
    if _p not in sys.path:
        sys.path.append(_p)

import ml_dtypes
import numpy as np

import concourse.mybir as mybir
from concourse import bass, bacc
from concourse.tile import TileContext
from concourse.bass_utils import run_bass_kernel_spmd

F32 = mybir.dt.float32
F16 = mybir.dt.float16
F8 = mybir.dt.float8e4
N_CORES = 8
B = 32
R = 2000
K = 20
NEG_BIG = -3.0e38
ALU = mybir.AluOpType
ACTF = mybir.ActivationFunctionType
AX = mybir.AxisListType

# (C, HW, channels-per-partition)
LAYERS = [(64, 3136, 1), (128, 784, 1), (256, 196, 2), (512, 49, 4)]

# --- engine assignment knobs -----------------------------------------------
# L0: 16 pairs (2 samples x 64ch on 128 partitions, 3136 free cols each).
#   'A' ACT accum | 'V' DVE reduce | 'GV'/'GA' gpsimd halve + DVE/ACT tail
L0_ASSIGN = ["GV", "A", "GV", "A", "GV", "GV", "A", "GV",
             "A", "GV", "GV", "A", "GV", "A", "GV", "GV"]  # 10 GV, 6 A
# L1: 8 tiles [128, 4, 784]. 'V' | 'A' (4 ops) | 'GV'
L1_ASSIGN = ["V", "A", "V", "A", "V", "GV", "A", "A",
             "V", "A", "V", "GV", "V", "A", "V", "A"]  # 7V, 7A, 2GV
# L2: 8 tiles [128, 4, 2, 196]. 'V' | 'GV'
L2_ASSIGN = ["GV", "GV", "GV", "GV", "GV", "GV", "GV", "GV"]
# L3: 8 tiles [128, 4, 4, 49]. 'V' | 'GV'
L3_ASSIGN = ["V", "GV", "V", "GV", "V", "GV", "V", "GV"]
# squares, keyed (layer, chunk_offset): 'A' | 'V' | 'G'
SQ_EARLY = [(2, 0), (2, 128), (3, 0), (3, 128), (3, 256), (3, 384)]
SQ_LATE = [(0, 0), (1, 0)]

RCHUNKS = [(0, 512), (512, 512), (1024, 512), (1536, 464)]
HALF_A = (0, 1024)
HALF_B = (1024, 976)


def build_nc():
    nc = bacc.Bacc("TRN2", target_bir_lowering=False, debug=False,
                   num_devices=N_CORES)

    feats = [nc.dram_tensor(f"feat{l}", [B, C, HW], F8, kind="ExternalInput")
             for l, (C, HW, _) in enumerate(LAYERS)]
    refTs = [nc.dram_tensor(f"refT{l}", [C, R], F16, kind="ExternalInput")
             for l, (C, _, _) in enumerate(LAYERS)]
    regw = nc.dram_tensor("regw", [1, 4], F32, kind="ExternalInput")
    regb = nc.dram_tensor("regb", [1, 1], F32, kind="ExternalInput")
    out = nc.dram_tensor("out", [B, 1], F32, kind="ExternalOutput")

    with TileContext(nc) as tc:
        with (
            tc.tile_pool(name="persist", bufs=1) as pp,
            tc.tile_pool(name="ft0", bufs=5) as fp0,
            tc.tile_pool(name="ft1", bufs=16) as fp1,
            tc.tile_pool(name="ft23", bufs=8) as fp23,
            tc.tile_pool(name="half", bufs=4) as hvp,
            tc.tile_pool(name="pbig", bufs=1,
                         space=bass.MemorySpace.PSUM) as pbig,
            tc.tile_pool(name="psmall", bufs=1,
                         space=bass.MemorySpace.PSUM) as psml,
        ):
            # ---------------- persistent tiles
            rt = {}
            sq = {}
            for l, (C, _, _) in enumerate(LAYERS):
                for i in range(0, C, 128):
                    Cc = min(128, C - i)
                    rt[(l, i)] = pp.tile([Cc, R], F16, tag=f"rt{l}_{i}",
                                         name=f"rt{l}_{i}")
                    sq[(l, i)] = pp.tile([Cc, R], F16, tag=f"sq{l}_{i}",
                                         name=f"sq{l}_{i}")
            sums0 = pp.tile([128, 16], F32, tag="sums0", name="sums0")
            sums1 = pp.tile([128, B], F32, tag="sums1", name="sums1")
            sums2 = pp.tile([128, B, 2], F32, tag="sums2", name="sums2")
            sums3 = pp.tile([128, B, 4], F32, tag="sums3", name="sums3")
            qT0 = pp.tile([64, 16, 2], F16, tag="qT0", name="qT0")
            qT1 = pp.tile([128, B], F16, tag="qT1", name="qT1")
            qT23 = {}
            for l, nch in ((2, 2), (3, 4)):
                for j in range(nch):
                    qT23[(l, j)] = pp.tile([128, 64], F16, tag=f"qT{l}_{j}",
                                           name=f"qT{l}_{j}")
            ones_col = pp.tile([128, 1], F16, tag="ones_col", name="ones_col")
            ones_row = pp.tile([1, B], F32, tag="ones_row", name="ones_row")
            # fold lhsT masks: row usage via column slices
            neg_full = pp.tile([128, 64], F16, tag="neg_full", name="neg_full")
            neg_l2 = pp.tile([128, 64], F16, tag="neg_l2", name="neg_l2")
            neg_l3 = pp.tile([128, 64], F16, tag="neg_l3", name="neg_l3")
            wb_sb = pp.tile([1, 5], F32, tag="wb_sb", name="wb_sb")
            vals48 = pp.tile([128, 48], F32, tag="vals48", name="vals48")
            vals = pp.tile([128, 24], F32, tag="vals", name="vals")
            fixrow = pp.tile([128, 1], F32, tag="fixrow", name="fixrow")
            scr0 = pp.tile([128, 3136], F8, tag="scr0", name="scr0")
            scr0b = pp.tile([128, 1568], F16, tag="scr0b", name="scr0b")
            scr1 = pp.tile([128, 784], F8, tag="scr1", name="scr1")

            nc.vector.memset(ones_col[:], 1.0)
            nc.vector.memset(ones_row[:], 1.0)
            nc.vector.memset(neg_full[:], -1.0)
            nc.vector.memset(neg_l2[:, 0:32], -1.0)
            nc.vector.memset(neg_l2[:, 32:64], 0.0)
            nc.vector.memset(neg_l3[:, 0:32], 0.0)
            nc.vector.memset(neg_l3[:, 32:64], -1.0)
            for l, nch in ((2, 2), (3, 4)):
                zs = slice(32, 64) if l == 2 else slice(0, 32)
                for j in range(nch):
                    nc.vector.memset(qT23[(l, j)][:, zs], 0.0)
            nc.sync.dma_start(out=wb_sb[0:1, 0:4], in_=regw[:])
            nc.sync.dma_start(out=wb_sb[0:1, 4:5], in_=regb[:])

            # ---------------- DMAs: L2/L3 + refs first, then L0, then L1
            # (small layers pool early so their long distance-matmul chains
            # overlap L0/L1 pooling; L0/L1 dists at the end are short)
            ref_order = [(3, 0), (3, 128), (3, 256), (3, 384),
                         (2, 0), (2, 128), (1, 0), (0, 0)]
            ftiles = {}

            def dma_feat(l, t):
                C, HW, cpp = LAYERS[l]
                nS = 2 if l in (0, 1) else 4
                if l == 0:
                    tile = fp0.tile([128, nS, HW], F8, tag="f0",
                                    name=f"f0_{t}")
                    src = bass.AP(feats[0], 4 * t * C * HW,
                                  [[HW, 128], [2 * C * HW, nS], [1, HW]])
                else:
                    fpool = fp1 if l == 1 else fp23
                    tile = fpool.tile([128, nS, cpp, HW], F8, tag=f"f{l}",
                                      name=f"f{l}_{t}")
                    src = bass.AP(feats[l], nS * t * C * HW,
                                  [[cpp * HW, 128], [C * HW, nS],
                                   [HW, cpp], [1, HW]])
                nc.sync.dma_start(out=tile[:], in_=src)
                ftiles[(l, t)] = tile

            for t in range(8):
                dma_feat(2, t)
                li = ref_order[t]
                nc.sync.dma_start(
                    out=rt[li][:],
                    in_=refTs[li[0]][li[1]:li[1] + rt[li].shape[0], :])
            for t in range(8):
                dma_feat(3, t)
            for t in range(8):
                dma_feat(0, t)
                dma_feat(1, 2 * t)
                dma_feat(1, 2 * t + 1)

            # ---------------- squares (sq = rt^2, fp16) on ACT
            # 6 early chunks fill ACT's idle window before L0 tiles arrive
            for (l, i) in SQ_EARLY:
                nc.scalar.square(sq[(l, i)][:], rt[(l, i)][:])

            # ---------------- fold matmuls first: big = -sum_c r_c^2
            # (depend only on refs/squares -> PE busy during pooling)
            big = pbig.tile([128, R], F32, tag="big", name="big")
            for c0, n in RCHUNKS:
                cs = slice(c0, c0 + n)
                for j in range(4):
                    i = 128 * j
                    nc.tensor.matmul(big[64:128, cs], neg_l3[:],
                                     sq[(3, i)][:, cs],
                                     start=(j == 0), stop=False)
                for j in range(2):
                    i = 128 * j
                    nc.tensor.matmul(big[64:128, cs], neg_l2[:],
                                     sq[(2, i)][:, cs],
                                     start=False, stop=False)
                nc.tensor.matmul(big[32:64, cs], neg_full[:, 0:32],
                                 sq[(1, 0)][:, cs], start=True, stop=False)
                nc.tensor.matmul(big[0:32, cs], neg_full[0:64, 0:32],
                                 sq[(0, 0)][:, cs], start=True, stop=False)

            # ---------------- pooling: L2, L3 first, then L0, then L1
            # L2 / L3
            for l in (2, 3):
                C, HW, cpp = LAYERS[l]
                assign = L2_ASSIGN if l == 2 else L3_ASSIGN
                sums = sums2 if l == 2 else sums3
                for t in range(8):
                    tile = ftiles[(l, t)]
                    eng = assign[t]
                    dst = sums[:, 4 * t:4 * t + 4, :]
                    if eng == "V":
                        nc.vector.tensor_reduce(dst, tile[:], axis=AX.X,
                                                op=ALU.add)
                    else:
                        hw2 = HW // 2  # 98 | 24 (49 odd: copy tail col)
                        if l == 2:
                            h = hvp.tile([128, 4, cpp, hw2], F16,
                                         tag=f"h{l}", name=f"h{l}_{t}")
                            nc.gpsimd.tensor_tensor(
                                h[:], tile[:, :, :, 0:hw2],
                                tile[:, :, :, hw2:HW], op=ALU.add)
                            nc.vector.tensor_reduce(dst, h[:], axis=AX.X,
                                                    op=ALU.add)
                        else:
                            h = hvp.tile([128, 4, cpp, hw2 + 1], F16,
                                         tag=f"h{l}", name=f"h{l}_{t}")
                            nc.gpsimd.tensor_tensor(
                                h[:, :, :, 0:hw2], tile[:, :, :, 0:hw2],
                                tile[:, :, :, hw2:2 * hw2], op=ALU.add)
                            nc.gpsimd.tensor_copy(h[:, :, :, hw2:hw2 + 1],
                                                  tile[:, :, :, 2 * hw2:HW])
                            nc.vector.tensor_reduce(dst, h[:], axis=AX.X,
                                                    op=ALU.add)
                for j in range(cpp):
                    nc.vector.tensor_scalar_mul(
                        qT23[(l, j)][:, 0:32] if l == 2
                        else qT23[(l, j)][:, 32:64],
                        sums[:, :, j], 2.0 / HW)

            # ---------------- quadrant-2 distances (overlap L0/L1 pooling)
            def dist_q2(c0, n, stop):
                cs = slice(c0, c0 + n)
                for l, nch in ((2, 2), (3, 4)):
                    for j in range(nch):
                        nc.tensor.matmul(
                            big[64:128, cs], qT23[(l, j)][:],
                            rt[(l, 128 * j)][:, cs], start=False,
                            stop=(stop and l == 3 and j == nch - 1))

            for c0, n in RCHUNKS:
                dist_q2(c0, n, True)

            # late squares (needed only by Q0/Q1 folds near the end)
            for (l, i) in SQ_LATE:
                nc.scalar.square(sq[(l, i)][:], rt[(l, i)][:])

            # L0 pairs + L1 half-tiles, interleaved per DMA round
            C0, HW0, _ = LAYERS[0]
            C1, HW1, _ = LAYERS[1]

            def pool_l0(pr):
                tile = ftiles[(0, pr // 2)]
                u = pr % 2
                eng = L0_ASSIGN[pr]
                dst = sums0[:, pr:pr + 1]
                if eng == "A":
                    nc.scalar.activation(scr0[:, :], tile[:, u, :],
                                         ACTF.Copy, accum_out=dst)
                elif eng == "V":
                    nc.vector.tensor_reduce(dst, tile[:, u, :],
                                            axis=AX.X, op=ALU.add)
                else:
                    h = hvp.tile([128, HW0 // 2], F16, tag="h0",
                                 name=f"h0_{pr}")
                    nc.gpsimd.tensor_tensor(
                        h[:], tile[:, u, 0:HW0 // 2],
                        tile[:, u, HW0 // 2:HW0], op=ALU.add)
                    if eng == "GV":
                        nc.vector.tensor_reduce(dst, h[:], axis=AX.X,
                                                op=ALU.add)
                    else:
                        nc.scalar.activation(scr0b[:, :], h[:],
                                             ACTF.Copy, accum_out=dst)

            def pool_l1(t):
                tile = ftiles[(1, t)]
                eng = L1_ASSIGN[t]
                dst = sums1[:, 2 * t:2 * t + 2]
                if eng == "V":
                    nc.vector.tensor_reduce(dst, tile[:, :, 0, :],
                                            axis=AX.X, op=ALU.add)
                elif eng == "A":
                    for g in range(2):
                        nc.scalar.activation(scr1[:, :], tile[:, g, 0, :],
                                             ACTF.Copy,
                                             accum_out=dst[:, g:g + 1])
                else:
                    h = hvp.tile([128, 2, HW1 // 2], F16, tag="h1",
                                 name=f"h1_{t}")
                    nc.gpsimd.tensor_tensor(h[:], tile[:, :, 0, 0:HW1 // 2],
                                            tile[:, :, 0, HW1 // 2:HW1],
                                            op=ALU.add)
                    nc.vector.tensor_reduce(dst, h[:], axis=AX.X, op=ALU.add)

            for t in range(8):
                pool_l0(2 * t)
                pool_l0(2 * t + 1)
                pool_l1(2 * t)
                pool_l1(2 * t + 1)
            for h in range(2):
                nc.vector.tensor_scalar_mul(
                    qT0[:, :, h], sums0[64 * h:64 * h + 64, :], 2.0 / HW0)
            nc.vector.tensor_scalar_mul(qT1[:], sums1[:], 2.0 / HW1)
            for c0, n in RCHUNKS:
                cs = slice(c0, c0 + n)
                nc.tensor.matmul(big[0:32, cs], qT0[:], rt[(0, 0)][:, cs],
                                 start=False, stop=True)
                nc.tensor.matmul(big[32:64, cs], qT1[:], rt[(1, 0)][:, cs],
                                 start=False, stop=True)

            # ---------------- top-24 (single pass over psum)
            nc.vector.max(vals[:, 0:8], big[:])
            nc.vector.match_replace(big[:], vals[:, 0:8], big[:], NEG_BIG)
            nc.vector.max(vals[:, 8:16], big[:])
            nc.vector.match_replace(big[:], vals[:, 8:16], big[:], NEG_BIG)
            nc.vector.max(vals[:, 16:24], big[:])

            # ---------------- qn2 -> fixrow = -qn2 per (l, s) row
            qsq0 = pp.tile([64, 16, 2], F16, tag="qsq0", name="qsq0")
            qsq1 = pp.tile([128, B], F16, tag="qsq1", name="qsq1")
            nc.scalar.activation(qsq0[:], qT0[:], ACTF.Square, scale=0.5)
            nc.scalar.activation(qsq1[:], qT1[:], ACTF.Square, scale=0.5)
            qpsA = psml.tile([64, 1], F32, tag="qpsA", name="qpsA")
            nc.tensor.matmul(qpsA[0:32, :], qsq0[:], ones_col[0:64, 0:1],
                             start=True, stop=True)
            nc.tensor.matmul(qpsA[32:64, :], qsq1[:], ones_col[:, 0:1],
                             start=True, stop=True)
            qpsB = psml.tile([64, 1], F32, tag="qpsB", name="qpsB")
            first = True
            for l, nch in ((2, 2), (3, 4)):
                for j in range(nch):
                    qsq = pp.tile([128, 64], F16, tag=f"qsq{l}_{j}",
                                  name=f"qsq{l}_{j}")
                    nc.scalar.activation(qsq[:], qT23[(l, j)][:],
                                         ACTF.Square, scale=0.5)
                    nc.tensor.matmul(qpsB[:], qsq[:], ones_col[:, 0:1],
                                     start=first,
                                     stop=(l == 3 and j == nch - 1))
                    first = False
            for l in range(4):
                src = qpsA if l < 2 else qpsB
                r0 = 32 * (l % 2)
                nc.scalar.activation(fixrow[32 * l:32 * l + 32, :],
                                     src[r0:r0 + 32, :], ACTF.Copy,
                                     scale=-1.0)

            # ---------------- LID
            ln2 = pp.tile([128, 24], F32, tag="ln2", name="ln2")
            S = pp.tile([128, 1], F32, tag="S", name="S")
            denom = pp.tile([128, 1], F32, tag="denom", name="denom")
            lid = pp.tile([128, 1], F32, tag="lid", name="lid")
            nc.vector.tensor_scalar(vals[:], vals[:], fixrow[:], -1e-30,
                                    op0=ALU.add, op1=ALU.min)
            nc.scalar.activation(ln2[:], vals[:], ACTF.Ln, scale=-1.0)
            nc.vector.tensor_reduce(S[:], ln2[:, 1:21], axis=AX.X,
                                    op=ALU.add)
            nc.vector.tensor_scalar(denom[:], ln2[:, 20:21], -20.0, S[:],
                                    op0=ALU.mult, op1=ALU.add)
            # denom scaled by -1/(2K) during the fuse above would change S;
            # keep DVE reciprocal then fold -2K via the lid4 copies' source
            nc.vector.reciprocal(lid[:], denom[:])
            nc.vector.tensor_scalar_mul(lid[:], lid[:], -2.0 * K)

            # ---------------- regression + sigmoid
            lid4 = pp.tile([B, 4], F32, tag="lid4", name="lid4")
            for l in range(4):
                nc.vector.tensor_copy(lid4[:, l:l + 1],
                                      lid[32 * l:32 * l + 32, :])
            wps = psml.tile([B, 5], F32, tag="wps", name="wps")
            nc.tensor.matmul(wps[:], ones_row[:], wb_sb[:],
                             start=True, stop=True)
            wbc = pp.tile([B, 5], F32, tag="wbc", name="wbc")
            nc.scalar.copy(wbc[:], wps[:])
            prod = pp.tile([B, 4], F32, tag="prod", name="prod")
            nc.vector.tensor_tensor(prod[:], lid4[:], wbc[:, 0:4],
                                    op=ALU.mult)
            ssum = pp.tile([B, 1], F32, tag="ssum", name="ssum")
            nc.vector.tensor_reduce(ssum[:], prod[:], axis=AX.X, op=ALU.add)
            res = pp.tile([B, 1], F32, tag="res", name="res")
            nc.scalar.activation(res[:], ssum[:], ACTF.Sigmoid,
                                 bias=wbc[:, 4:5])
            nc.sync.dma_start(out=out[:], in_=res[:])

    nc.compile()
    return nc


_NC = None


def _get_nc():
    global _NC
    if _NC is None:
        _NC = build_nc()
    return _NC


def _perm(cpp, C):
    return [cpp * p + j for j in range(cpp) for p in range(C // cpp)]


def run(trace=False, **inputs):
    nc = _get_nc()
    feats = [np.asarray(inputs[f"feat{l}"], dtype=np.float32)
             for l in range(4)]
    refTs = []
    for l, (C, HW, cpp) in enumerate(LAYERS):
        rT = np.asarray(inputs[f"ref{l}"], dtype=np.float32).T  # [C, R]
        if cpp > 1:
            rT = rT[_perm(cpp, C)]
        refTs.append(np.ascontiguousarray(rT).astype(np.float16))
    regw = np.asarray(inputs["reg_w"], dtype=np.float32).reshape(1, 4)
    regb = np.asarray(inputs["reg_b"], dtype=np.float32).reshape(1, 1)
    assert int(inputs.get("k", K)) == K

    in_maps = []
    for c in range(N_CORES):
        m = {}
        for l, (C, HW, _) in enumerate(LAYERS):
            m[f"feat{l}"] = np.ascontiguousarray(
                feats[l][c * B:(c + 1) * B].reshape(B, C, HW)).astype(
                    ml_dtypes.float8_e4m3)
            m[f"refT{l}"] = refTs[l]
        m["regw"] = regw
        m["regb"] = regb
        in_maps.append(m)

    res = run_bass_kernel_spmd(nc, in_maps, core_ids=list(range(N_CORES)),
                               trace=trace)
    full = np.empty((N_CORES * B,), dtype=np.float32)
    for c in range(N_CORES):
        full[c * B:(c + 1) * B] = res.results[c]["out"][:, 0]
    return full, res


def kernel(**inputs):
    return run(trace=False, **inputs)[0]
